# revision 37
# baseline (speedup 1.0000x reference)
"""Trainium2 (8 NeuronCores) kernel for single-head causal attention.

Problem: x [8, 2048, 1024] f32; Wq/Wk/Wv [1024, 128] f32.
    q = x @ Wq ; k = x @ Wk ; v = x @ Wv          (per batch row)
    out = softmax(causal(q @ k^T / sqrt(128))) @ v  -> [8, 2048, 128] f32

Sharding: pure data-parallel — one batch row per NeuronCore, weights
replicated. No collectives.

Per-core algorithm (bf16 matmul inputs, f32 PSUM accumulation):
  Host supplies xT = x[b].T  [D, T] in bf16 (layout prep only).
  A) qT/kT [H=128 part, T] with W-chunks stationary over 8 D-chunks;
     v [T-block part, H] computed per k-block inside phase B. An all-ones
     column is appended to v so the softmax denominator falls out of the
     output matmul for free.
  B) Scores computed TRANSPOSED: sT[k,q] = kT_j-block-stationary @ qT,
     exact-causal (q in [j*128, T) per k-block j). exp(scale*s) runs on
     ScalarE straight out of PSUM into a CAUSAL-PACKED bf16 wT tile.
     No max-subtraction: |scale*s| <= ~7 here, safely in f32/bf16 range.
     Diagonal 128x128 blocks additionally get a multiplicative 0/1
     strictly-causal mask into separate diag tiles.
  C) out[q,h] accumulates over k-blocks j<=i with wT blocks stationary and
     v_aug moving (N=129). Column 128 of PSUM is l = sum_k w; normalize
     with one reciprocal + ScalarE copy-with-per-partition-scale, DMA out.

Input streaming (the v1 kernel lost ~7 us here): per-DMA throughput caps
well below the ~358 GB/s aggregate (full-width chunk solo ~92 GB/s), and
concurrent DMAs round-robin at packet granularity so with all 8 chunk
DMAs in flight chunk 0's completion fires only after ~60% of the input
transferred. v4: every chunk is partition-split into sub-DMAs across the
SP+ACT trigger rings, ~3 chunks in flight via Pool WAR-link chaining, so
completions arrive in consumption order at full aggregate bandwidth.

PE warm-up: HAM's clock governor ramps with sustained engine density
(~1.2 GHz at start, full clock after ~3.4 us of UNBROKEN work; gaps delay
the ramp and the whole kernel runs slow). Dummy matmuls bridge the
launch/DMA dead window so the PE never idles from first dispatch on.

Engine discipline: hardware compute instructions carry at most ONE
semaphore wait (bacc's legalization splits the rest into event-semaphore
junctions; redundant same-engine self-waits are stripped post-build —
PE/ACT/DVE complete strictly in order). Tile tracks dependencies at
subtile granularity. exp and the final per-row scale run on ScalarE;
per-partition-scalar multiplies must use ScalarE activation scale APs
(DVE tensor_scalar AP / stride-0 broadcasts give wrong results on
hardware). The A->B junction interleaves q/k stop-matmuls with half-tile
PSUM->SBUF copies (qT halves on DVE, kT halves on ACT) so the first score
matmul's deps are ready the cycle the last stop-matmul retires.
"""

from contextlib import ExitStack

import ml_dtypes
import numpy as np

B, T, D, H = 8, 2048, 1024, 128
P = 128
DC = D // P  # 8 contraction chunks
TB = T // P  # 16 token blocks
QG = T // 512  # 4 512-wide token groups
SCALE = 1.0 / float(np.sqrt(H))

_CACHE = {}
LAST_RESULT = None


def _build():
    import concourse.bacc as bacc
    import concourse.mybir as mybir
    import concourse.tile as tile

    f32 = mybir.dt.float32
    bf16 = mybir.dt.bfloat16
    EXP = mybir.ActivationFunctionType.Exp
    MULT = mybir.AluOpType.mult
    ADD = mybir.AluOpType.add

    nc = bacc.Bacc()
    # x host-prelayouted as [group g, half h, p, c-in-half, col]: one
    # contiguous 512 KB DMA per (g,h) half, dims matching the SBUF AP
    # xT_sb[:, 4h:4h+4, g*512:(g+1)*512] exactly. 8 big ordered transfers
    # (pitch-matched to phase A's group-outer consumption) replace the 11+
    # per-chunk DMAs whose ~0.62 us/trigger serial dispatch dominated the
    # v5-v8 streams.
    xg_h = nc.declare_dram_parameter("xg", [QG, 2, P, 4, 512], bf16, isOutput=False)
    # Wq|Wk interleaved per d-chunk [c, p, 2H]: chunk c's weights are one
    # contiguous 64 KB DMA triggered just before xT chunk c, so the first
    # q matmul's gate is only wqk0+chunk0 (576 KB) instead of all of
    # Wq/Wk (v5 loaded 512 KB of weights ahead of the stream).
    wqk_h = nc.declare_dram_parameter("Wqk", [P, DC, 2 * H], bf16, isOutput=False)
    wv_h = nc.declare_dram_parameter("Wv", [P, DC, H], bf16, isOutput=False)
    mask_h = nc.declare_dram_parameter("mask", [P, P], bf16, isOutput=False)
    out_h = nc.declare_dram_parameter("out", [T, H], f32, isOutput=True)

    with tile.TileContext(nc) as tc:
        with ExitStack() as ctx:
            singles = ctx.enter_context(tc.tile_pool(name="singles", bufs=1))

            xT_sb = singles.tile([P, DC, T], bf16)
            wqk_sb = singles.tile([P, DC, 2 * H], bf16)
            wv_sb = singles.tile([P, DC, H], bf16)
            mask_sb = singles.tile([P, P], bf16)
            mask2_sb = singles.tile([P, P], bf16)
            qT_sb = singles.tile([P, T], bf16)
            kT_sb = singles.tile([P, T], bf16)
            v_sb = singles.tile([P, TB, 132], bf16)  # [...,128] = ones col
            wT_sb = singles.tile([P, 17408], bf16)  # causal-packed
            dw_sb = singles.tile([P, TB, P], bf16)  # masked diagonal blocks
            link_sb = singles.tile([P, DC + 2], bf16)  # DMA-chain dummies
            warm_sb = singles.tile([P, 512], bf16)
            # per-iteration epilogue slices (no pool recycling -> no WAR waits)
            rec_all = singles.tile([P, TB], f32)
            os_all = singles.tile([P, TB, P], f32)  # unnormalized staging
            ot_all = singles.tile([P, TB, P], f32)

            # warm-up fodder memset on Pool: its queue frees earliest
            # (~5.9 us) so the PE's first dummy can start ~7.3 us (v7 put
            # this on DVE, which frees ~7.4, and the PE start slipped).
            nc.gpsimd.memset(warm_sb, 0.0)
            nc.vector.memset(v_sb[:, :, 128:129], 1.0)
            # col 129 zero so the 130-wide output matmuls (even column count;
            # 129-col ones measured +24% over ideal) accumulate 0 there.
            nc.vector.memset(v_sb[:, :, 129:130], 0.0)

            # --- Input stream. Measured DMA law (see transcript probes): a
            # single full-width [128, 2048] DMA sustains only ~92 GB/s; 4
            # partition-subs of one chunk ~143; 4 concurrent full chunks
            # ~278; 8 concurrent ~333 (aggregate cap ~358). Per-DMA
            # throughput is the wall, so EVERY chunk is split into
            # partition-subs, and ~3 chunks (6 subs) stay in flight: chunk c
            # (c>=3) WAR-waits a Pool "link" op that RAW-waits chunk c-3's
            # completion. Completions then arrive IN CONSUMPTION ORDER every
            # ~1.4 us (all-concurrent DMAs round-robin at packet granularity,
            # which lands every completion at ~60% of the whole transfer —
            # v1 idled the PE 7.5 us waiting for chunk 0). Chunk 0 gets 4
            # subs (~143 GB/s solo) since its completion opens phase A.
            def link(read_done_ap, war_ap, li):
                # RAW on read_done_ap's producer; the next DMA (writing
                # war_ap's region) then WARs on this op -> its trigger fires
                # only after the producer's completion semaphore.
                nc.gpsimd.tensor_tensor(
                    link_sb[:, li : li + 1], read_done_ap, war_ap, ADD
                )

            def half_ap(g, h):
                return xT_sb[:, 4 * h : 4 * h + 4, g * 512 : (g + 1) * 512]

            def half_dma(g, h, eng):
                eng.dma_start(out=half_ap(g, h), in_=xg_h[g, h])

            # v9 stream. Trigger economics (measured): ~0.62 us serial
            # dispatch per dma_start on its queue engine, ~0.75 us trigger->
            # payload latency, no payload before ~7.0 us, Scalar blocked
            # until ~8.5 by ACT_TABLE_LOAD. Eight ordered 512 KB halves,
            # pitch ~1.55 us at aggregate bw vs phase A's 1.73 us/half
            # consumption. g2/g3 halves chain on g0/g1 completions (Pool
            # links) so late transfers can't round-robin-steal bandwidth
            # from the head (the v5-v8 failure mode).
            # The measured DMA law has three simultaneous constraints:
            #  - aggregate bw scales with STREAM COUNT (~92 GB/s for one
            #    transfer, ~330 only with 7+ concurrent) — v9.2's depth-2
            #    chain left 2 streams in flight and starved everything;
            #  - trigger dispatch is ~0.62 us serial per dma_start, so at
            #    most ~2 triggers per 1.73 us-of-consumption are sustainable;
            #  - fair round-robin service means completion ORDER follows
            #    per-stream size and start stagger, not trigger order.
            # Scheme: g0's halves as 4 subs each (12-stream head, lands
            # first), g1's as 2 subs each from Scalar's queue, g2/g3's as
            # 2 subs each chained depth-3 (trigger when the half three
            # back completes): 4-8 streams stay in flight, completions
            # arrive in consumption order every ~1.7 us.
            def sub_dma(g, h, nsub, eng):
                w = P // nsub
                for s in range(nsub):
                    eng.dma_start(
                        out=xT_sb[
                            w * s : w * (s + 1), 4 * h : 4 * h + 4,
                            g * 512 : (g + 1) * 512,
                        ],
                        in_=xg_h[g, h, w * s : w * (s + 1)],
                    )

            # Mirror of the proven v5 stream profile (A-start 13.1, one
            # bounded mid-stall), retargeted at the group-major layout:
            # 8-stream head (wqk partition-halves + g0h0 4-sub) dispatched
            # interleaved across Sync+Scalar (ring credits cap ~8-9
            # outstanding; a 3-piece wqk split landed its middle piece at
            # 17.5 us and stalled A for 3.9 us), then the 7 remaining
            # halves ride full-width unchained — completions burst-cluster
            # but the 0.65 us arrival pitch beats the 1.73 consumption.
            def g0h0_sub(s, eng):
                eng.dma_start(
                    out=xT_sb[32 * s : 32 * (s + 1), 0:4, 0:512],
                    in_=xg_h[0, 0, 32 * s : 32 * (s + 1)],
                )

            # wqk as 4x128 KB partition-subs AHEAD of g0h0: its full
            # residency gates the very first q matmul, and 256 KB halves
            # were the last finishers among 10 streams (A-start slipped to
            # 15.4 us).
            nc.sync.dma_start(out=wqk_sb[0:32], in_=wqk_h[0:32])
            nc.scalar.dma_start(out=wqk_sb[32:64], in_=wqk_h[32:64])
            g0h0_sub(0, nc.sync)
            g0h0_sub(1, nc.scalar)
            nc.sync.dma_start(out=wqk_sb[64:96], in_=wqk_h[64:96])
            nc.scalar.dma_start(out=wqk_sb[96:128], in_=wqk_h[96:128])
            g0h0_sub(2, nc.sync)
            g0h0_sub(3, nc.scalar)
            halves = [(0, 1), (1, 0), (1, 1), (2, 0), (2, 1), (3, 0), (3, 1)]
            for i, (g, h) in enumerate(halves):
                half_dma(g, h, nc.sync if i % 2 == 0 else nc.scalar)
            link(half_ap(2, 0)[:, 3, 511:512], wv_sb[:, 0, 0:1], 0)
            nc.sync.dma_start(out=wv_sb, in_=wv_h[:])
            link(half_ap(2, 1)[:, 3, 511:512], mask_sb[:, 0:1], 1)
            nc.scalar.dma_start(out=mask_sb, in_=mask_h[:])
            # (the mask2 ACT pre-touch copy is emitted mid-phase-A, after
            # junction_g(1), so it never blocks the kT copies or exps)

            # --- Phase A1: q/k projections, d-chunk OUTER so each xT chunk is
            # consumed as its DMA lands (PE overlaps the input load). 8 PSUM
            # banks live at once; pool scoped so phase B/C reuse the space.
            with tc.tile_pool(name="psQK", bufs=1, space="PSUM") as psQK:
                # ONE persistent PSUM pool, SIX tiles, for the WHOLE
                # kernel: closing the A-pool and opening B-pools put a
                # release->alloc barrier (gated on ALL of A's copies) before
                # the first score matmul — ~1.2 us. Phase B's buffers ARE
                # phase A's accumulators with clean per-TILE dependencies:
                #   S0/S1 [P,1024]: q accum (A) -> score ping-pong (B)
                #   kps[0] [P,512]: k group 0 (A) -> v-projection (B)
                #   kps[1..3]:      k groups 1-3 (A) -> 3 output slots (B)
                S0 = psQK.tile([P, 1024], f32, tag="S0")
                S1 = psQK.tile([P, 1024], f32, tag="S1")
                kps = [
                    psQK.tile([P, 512], f32, tag=f"kps{g}", name=f"kps{g}")
                    for g in range(QG)
                ]
                # PE warm-up fodder: the PE sequencer starts ~8.1 us (its
                # ~770-instruction queue loads last); 12 x 512 dummies end
                # ~13.2 us, dovetailing with chunk 0's completion. HAM's
                # clock governor ramps with engine DENSITY — v3's sparse
                # early stream delayed full clock to 25.7 us and the WHOLE
                # kernel ran ~15% slow — so the PE must never idle from
                # warm-up start onward. They write qps[0] BEFORE its real
                # accumulation group begins (start=True clears the bank).
                for _ in range(13):
                    nc.tensor.matmul(
                        S0[:, 0:512], warm_sb[:, 0:128], warm_sb,
                        start=True, stop=True,
                    )

                def qk_mm(is_q, g, c):
                    w_ap = (
                        wqk_sb[:, c, 0:H] if is_q else wqk_sb[:, c, H : 2 * H]
                    )
                    if is_q:
                        acc = (S0 if g < 2 else S1)[
                            :, (g % 2) * 512 : (g % 2) * 512 + 512
                        ]
                    else:
                        acc = kps[g]
                    nc.tensor.matmul(
                        acc,
                        w_ap,
                        xT_sb[:, c, g * 512 : (g + 1) * 512],
                        start=(c == 0),
                        stop=(c == DC - 1),
                    )

                def junction_g(g):
                    # group g's stops just retired: stream its copies out.
                    # kT on ACT, qT halves on DVE; ACT's FIFO becomes
                    # [kT0, kT1, mask2, exp..., kT2, exp..., kT3, ...] so
                    # no exp ever sits behind more than one 0.6 us copy.
                    ksrc, qsrc = kps[g], (S0 if g < 2 else S1)
                    nc.scalar.copy(
                        kT_sb[:, g * 512 : (g + 1) * 512], ksrc
                    )
                    nc.vector.tensor_copy(
                        qT_sb[:, g * 512 : (g + 1) * 512],
                        qsrc[:, (g % 2) * 512 : (g % 2) * 512 + 512],
                    )

                # --- Phases B+A2+C, software-pipelined by one j: per k-block
                # j emit its score matmuls + exp + v projection, then output
                # group C_{j-1}, whose inputs (exps/dw/v for blocks <= j-1)
                # are all complete by then — so C's matmuls carry no waits and
                # the PE stream stays dense while ScalarE exps run alongside.
                out_ap = out_h[:]

                # Causal-packed wT layout: segment for k-block j holds
                # q in [j*128, T) at packed offset OFF[j]; segments are
                # back-to-back so exp runs in maximal 1024-wide ops across
                # block boundaries (ACT op overhead is ~352 cycles each).
                OFF = [0] * (TB + 1)
                for j in range(TB):
                    OFF[j + 1] = OFF[j] + (T - j * P)
                TOTAL = OFF[TB]  # 17408

                def wT_at(jj, qstart, width):
                    o = OFF[jj] + (qstart - jj * P)
                    return wT_sb[:, o : o + width]

                def emit_c_group(i):
                    # 130-wide (even) moving side: col 128 = ones (denom),
                    # col 129 = zeros pad; odd 129-col matmuls measured
                    # 66.8 ns vs the 53.75 ideal, 130 should pipeline clean.
                    po = kps[1 + i % 3][:, 0:132]
                    for jj in range(i):
                        nc.tensor.matmul(
                            po[:, 0:130],
                            wT_at(jj, i * P, P),
                            v_sb[:, jj, 0:130],
                            start=(jj == 0),
                            stop=False,
                        )
                    nc.tensor.matmul(
                        po[:, 0:130],
                        dw_sb[:, i, :],
                        v_sb[:, i, 0:130],
                        start=(i == 0),
                        stop=True,
                    )
                    nc.vector.reciprocal(rec_all[:, i : i + 1], po[:, 128:129])
                    # per-partition normalize OFF ScalarE: with it there,
                    # ACT's B-phase ledger (17 exps ~17us + 16 muls ~7us)
                    # exceeded the PE's ~22us window and became co-critical.
                    # DVE's tensor_scalar reads PSUM WRONG on hardware (rel
                    # err 19!) but is correct from SBUF (probe), and Pool's
                    # tensor_scalar takes 2us/op (slow DSP): so DVE copies
                    # PSUM->SBUF (its usual, correct path), then multiplies
                    # in SBUF, both on DVE.
                    # The LAST two groups normalize on ScalarE directly from
                    # PSUM (its exps are done by then; the activation-scale
                    # path is PSUM-correct) — one hop fewer on the tail.
                    if i >= TB - 2:
                        nc.scalar.mul(
                            ot_all[:, i, :], po[:, 0:H], rec_all[:, i : i + 1]
                        )
                    else:
                        nc.vector.tensor_copy(os_all[:, i, :], po[:, 0:H])
                        nc.vector.tensor_scalar_mul(
                            ot_all[:, i, :], os_all[:, i, :], rec_all[:, i : i + 1]
                        )
                    if i == TB - 1:
                        # tail DMA split in partition halves on two queues
                        # (each row is a contiguous 512 B DRAM write, no
                        # sub-granule penalty) to halve the last transfer.
                        nc.scalar.dma_start(
                            out=out_ap[i * P : i * P + 64, :],
                            in_=ot_all[0:64, i, :],
                        )
                        nc.sync.dma_start(
                            out=out_ap[i * P + 64 : (i + 1) * P, :],
                            in_=ot_all[64:128, i, :],
                        )
                    else:
                        nc.sync.dma_start(
                            out=out_ap[i * P : (i + 1) * P, :],
                            in_=ot_all[:, i, :],
                        )

                def emit_j_epilogue(j):
                    # diag mask (on Pool — all-SBUF, keeps DVE for the
                    # PSUM-side copies), v projection, pipelined output
                    # group. The LAST diag multiply sits on the kernel's
                    # critical tail: DVE's 190 ns beats Pool's 405 ns there.
                    eng = nc.vector if j == TB - 1 else nc.gpsimd
                    eng.tensor_tensor(
                        dw_sb[:, j, :], wT_at(j, j * P, P), mask2_sb, MULT
                    )
                    # v PSUM ping-pong between two 128-col slots of kps[0]:
                    # a single slot made block j's first matmul WAR-wait on
                    # block j-1's PSUM->SBUF cast (~0.4 us bubble per block).
                    pv = kps[0][:, (j % 3) * H : (j % 3) * H + H]
                    for c in range(DC):
                        nc.tensor.matmul(
                            pv,
                            xT_sb[:, c, j * P : (j + 1) * P],
                            wv_sb[:, c, :],
                            start=(c == 0),
                            stop=(c == DC - 1),
                        )
                    nc.vector.tensor_copy(v_sb[:, j, 0:H], pv)
                    if j > 0:
                        emit_c_group(j - 1)

                def emit_scores(j, q0, q1, ps):
                    # score matmuls for segment j, q in [q0, q1), into
                    # ps[:, 0:q1-q0]; split at the 512 PSUM bank boundary
                    a = q0
                    while a < q1:
                        b = min(q1, a + 512 - (a - q0) % 512)
                        nc.tensor.matmul(
                            ps[:, a - q0 : b - q0],
                            kT_sb[:, j * P : (j + 1) * P],
                            qT_sb[:, a : a + (b - a)],
                            start=True,
                            stop=True,
                        )
                        a = b

                def emit_exp(j, q0, q1, ps):
                    nc.scalar.activation(
                        wT_at(j, q0, q1 - q0), ps[:, : q1 - q0], EXP,
                        scale=SCALE,
                    )

                # --- Phase A, group-outer: group g's q/k complete right
                # after its xT columns land; copies stream out mid-A. After
                # g1, qT[0:1024]/kT[0:1024] exist and S0 is free: the
                # score+exp PRE-RUN for every segment's q<1024 portion
                # (4608 packed cols, 8 partial-segment chunks, serially
                # through S0) slides INTO phase A's g2/g3 window, pulling
                # ~5 us of ACT exp work off phase B's critical path.
                pre = [(j, j * P, 1024) for j in range(8)]

                def sweep(g, h):
                    for c in range(4 * h, 4 * h + 4):
                        qk_mm(True, g, c)
                        qk_mm(False, g, c)

                # Interleave discipline: each pre-run score chunk is placed
                # at least one half-sweep (1.7 us) after the exp whose S0
                # region it WAR-overwrites, so the in-order PE never stalls
                # on ACT inside phase A.
                sweep(0, 0)
                sweep(0, 1)
                junction_g(0)
                sweep(1, 0)
                sweep(1, 1)
                junction_g(1)
                nc.scalar.copy(mask2_sb, mask_sb)
                sweep(2, 0)
                emit_scores(*pre[0], S0)
                emit_exp(*pre[0], S0)
                sweep(2, 1)
                junction_g(2)
                emit_scores(*pre[1], S0)
                emit_exp(*pre[1], S0)
                sweep(3, 0)
                emit_scores(*pre[2], S0)
                emit_exp(*pre[2], S0)
                sweep(3, 1)
                junction_g(3)
                emit_scores(*pre[3], S0)
                emit_exp(*pre[3], S0)
                # catch-up: drain pre-run exps 4-7 with epilogues 0-3
                # interleaved (their deps — diag exps, xT g0, wv, freed
                # kps banks — are all met) so the PE stays dense while
                # ACT finishes the pre-run ledger.
                emit_j_epilogue(0)
                emit_j_epilogue(1)
                emit_scores(*pre[4], S0)
                emit_exp(*pre[4], S0)
                emit_j_epilogue(2)
                emit_scores(*pre[5], S0)
                emit_exp(*pre[5], S0)
                emit_j_epilogue(3)
                emit_scores(*pre[6], S0)
                emit_exp(*pre[6], S0)
                emit_scores(*pre[7], S0)
                emit_exp(*pre[7], S0)

                # --- Phase B: remaining packed region = every segment's
                # q >= 1024 remainder (segs 0-7: exactly 1024 cols each;
                # segs 8-15: their full spans). 1024-wide chunks amortize
                # ACT's ~352-cycle fixed cost; the tail goes finer, split
                # at segment boundaries, so the final exp is only seg15's
                # 128-col head. Epilogues: segs 0-7 are pre-unblocked
                # (drained at most 2 per chunk so the PE stream between
                # exps stays even); 8-15 gate on their diag head's exp.
                work = [(j, 1024, T) for j in range(8)]
                for j, a, bnd in (
                    (8, 1024, 2048), (9, 1152, 2048), (10, 1280, 2048),
                    (11, 1408, 2048), (12, 1536, 2048), (13, 1664, 2048),
                    (14, 1792, 2048), (15, 1920, 2048),
                ):
                    work.append((j, a, bnd))
                head_ready = [True] * 8 + [False] * 8
                next_done = 4  # epilogues 0-3 drained in the catch-up
                for ci, (j, q0, q1) in enumerate(work):
                    ps = S1 if ci % 2 == 0 else S0
                    emit_scores(j, q0, q1, ps)
                    emit_exp(j, q0, q1, ps)
                    if j >= 8:
                        head_ready[j] = True
                    emitted = 0
                    while (
                        next_done < TB
                        and head_ready[next_done]
                        and emitted < 2
                    ):
                        emit_j_epilogue(next_done)
                        next_done += 1
                        emitted += 1
                while next_done < TB:
                    emit_j_epilogue(next_done)
                    next_done += 1
                emit_c_group(TB - 1)

    _strip_self_waits(nc)
    nc.finalize()  # Bacc.compile(): wait legalization + register allocation
    return nc


def _strip_self_waits(nc):
    """Drop same-engine semaphore waits on in-order engines (PE/ACT/DVE
    execute and complete strictly in order, so a self-wait is redundant).
    Tile emits them conservatively; walrus allows only one sem wait per
    compute instruction, and these push some matmuls/tensor-ops over."""
    prefixes = {"PE": "PE_", "Activation": "Activation_", "DVE": "DVE_"}
    for bb in nc.m.functions[0].blocks:
        for inst in bb.instructions:
            si = inst.sync_info
            if not si or not si.on_wait:
                continue
            pref = prefixes.get(str(inst.engine).split(".")[-1])
            if pref is None:
                continue
            keep = [w for w in si.on_wait if not (w.ant_name or "").startswith(pref)]
            if len(keep) != len(si.on_wait):
                si.on_wait = keep
                inst.sync_info = si


def kernel(**inputs):
    global LAST_RESULT
    x = np.asarray(inputs["x"], dtype=np.float32)
    bf = ml_dtypes.bfloat16
    # Wq|Wk interleaved per d-chunk: [DC, P, 2H], one contiguous 64 KB DMA
    # per chunk. Wv keeps the [P, DC, H] layout (single trailing DMA).
    wq_c = np.asarray(inputs["Wq"], dtype=np.float32).astype(bf).reshape(DC, P, H)
    wk_c = np.asarray(inputs["Wk"], dtype=np.float32).astype(bf).reshape(DC, P, H)
    wqk = np.ascontiguousarray(
        np.concatenate([wq_c, wk_c], axis=2).transpose(1, 0, 2)
    )
    wv_bf = np.ascontiguousarray(
        np.asarray(inputs["Wv"], dtype=np.float32)
        .astype(bf)
        .reshape(DC, P, H)
        .transpose(1, 0, 2)
    )
    # dw[p=k_local, f=q_local] keeps entries with k <= q
    mask01 = (
        (np.arange(P)[:, None] <= np.arange(P)[None, :]).astype(np.float32).astype(bf)
    )

    if "nc" not in _CACHE:
        _CACHE["nc"] = _build()
    nc = _CACHE["nc"]

    from concourse.bass_utils import run_bass_kernel_spmd

    in_maps = [
        {
            # [g, h, p, c_in_half, col]: d = h*512 + c*128 + p, t = g*512+col
            "xg": np.ascontiguousarray(
                x[b]
                .T.astype(bf)
                .reshape(2, 4, P, QG, 512)
                .transpose(3, 0, 2, 1, 4)
            ),
            "Wqk": wqk,
            "Wv": wv_bf,
            "mask": mask01,
        }
        for b in range(B)
    ]
    res = run_bass_kernel_spmd(nc, in_maps, core_ids=list(range(B)))
    LAST_RESULT = res
    return np.stack([res.results[b]["out"] for b in range(B)]).astype(np.float32)

